# revision 53
# baseline (speedup 1.0000x reference)
"""Sparse BertSelfAttention on 8 trn2 NeuronCores.

Sharding: core c -> batch b = c//4, head-group g = c%4 (heads 4g..4g+3).
Each core computes its batch's QT/KT/V projections for its 4 heads and the
sparse attention (local 128-band + global summary columns), producing the
output column block [2048, 256] for its (batch, head-group).

Sparse structure (STRIDE=128, EXPR=8, L=2048, bidirectional):
  Query q's local key block is b*(q) = (q-1)//128 (q>=1); b*(0) = 0.
  So the SHIFTED query window w = [128w+1, 128w+129) attends key block w
  fully dense (the baseline's separate "prev-block" path disappears), plus
  the global summary columns:
    A: cols with (c mod 128) in 120..127  (128 cols)
    B: cols 128, 256, ..., 1920           (15 cols)
  The "+1 col" of each local window is always in set B. Because each window
  maps to ONE key block, the double-count mask (global cols excluded from
  the local tile) depends only on the key index within the block, i.e. it is
  a per-PARTITION bias folded into the Exp activation -- no mask tensors,
  no mask matmuls. Window 0 allows key col 0 (not in B): fixed by re-running
  exp on the single k=0 partition strip. Query row 0 (keys = block 0 +
  globals) gets a tiny dedicated path (computed 8 queries wide because
  column-tiled PE modes with narrow rhs hang the hardware).

Layout trick (as baseline): scores are computed transposed (S^T[k, q], keys
on partitions) so softmax denominators come from a ones-column appended to
V, and P @ V is computed with lhsT = P^T directly (no transposes anywhere).
exp() skips max-subtraction: allowed scores are O(5), masked underflow to 0.

Projections stream ht chunk-outer (c-outer) against 8 concurrent PSUM
accumulators so the PE starts as soon as the first 128-row chunk of ht
lands (each chunk's slice of Wk is packed in front of it so one DMA +
one semaphore covers both); warmup matmuls run during the initial DMA
wait so real matmuls execute at full PE clock (p-state ramp).
"""

import numpy as np
import ml_dtypes

import concourse.bass as bass
from concourse import bacc
import concourse.mybir as mybir
import concourse.tile as tile
from concourse.bass_utils import run_bass_kernel_spmd

BF16 = mybir.dt.bfloat16
F32 = mybir.dt.float32
AF = mybir.ActivationFunctionType

L = 2048
HID = 1024
NB = L // 128  # 16 key blocks / query windows
NEG = -10000.0

_prog_cache = {}


def _rep_attnB(v):
    out = np.zeros((128, 1), np.float32)
    for h in range(4):
        out[32 * h : 32 * h + 15, 0] = v
    return out


def _glob_cols():
    # A: (16 blocks) x (8 cols 120..127); B: 128,256,...,1920
    a = (np.arange(16)[:, None] * 128 + 120 + np.arange(8)[None, :]).reshape(-1)
    b = np.arange(1, 16) * 128
    return a, b


def build_program(loop_n=None, am_zero=True, act_dma=True, c_outer=True,
                  skip_q0=False, skip_strip=False, q0_level=6):
    nc = bacc.Bacc(None)
    # ht chunks with the matching wk chunk packed in front: [wk 256 | ht 2048]
    ht_d = nc.dram_tensor("htwk", [HID, 2304], BF16, kind="ExternalInput")
    wq_d = nc.dram_tensor("wq", [HID, 256], BF16, kind="ExternalInput")
    wv_d = nc.dram_tensor("wv", [HID, 260], BF16, kind="ExternalInput")
    # smalls: cols 0:4 bqk | 4:5 attnA | 5:6 attnB | 6:22 biasS
    sm_d = nc.dram_tensor("smalls", [128, 6 + NB], F32, kind="ExternalInput")
    out_d = nc.dram_tensor("out", [L, 256], F32, kind="ExternalOutput")

    with tile.TileContext(nc) as tc:
        with (
            tc.tile_pool(name="consts", bufs=1) as consts,
            tc.tile_pool(name="pp", bufs=8) as pp,
            tc.tile_pool(name="po", bufs=8) as po,
            tc.tile_pool(name="psmall", bufs=8) as psmall,
            tc.tile_pool(name="ps", bufs=1, space="PSUM") as psum,
        ):
            import contextlib
            _lp = tc.For_i(0, loop_n, 1) if loop_n else contextlib.nullcontext()
            with _lp:
                # ---- input DMAs, split across the two HWDGE queues ----
                # SP queue: ht0, wq, ht2, ht4, ht6, wv, bqk, aA
                # Act queue: wk, ht1, ht3, ht5, ht7, biasS, aB
                hwl = [
                    consts.tile([128, 2304], BF16, tag=f"hw{c}", name=f"hw{c}")
                    for c in range(8)
                ]
                htl = [hw[:, 256:2304] for hw in hwl]
                wq = consts.tile([128, 8, 256], BF16)
                wv = consts.tile([128, 8, 260], BF16)
                smalls = consts.tile([128, 6 + NB], F32)
                bqk = smalls[:, 0:4]
                aA = smalls[:, 4:5]
                aB = smalls[:, 5:6]
                bS = smalls[:, 6 : 6 + NB]

                q2 = nc.scalar if act_dma else nc.sync
                # one packed [wk|ht] transfer per contraction chunk, split in
                # halves aligned to the K-pass consumption order; all big
                # transfers on the SP queue in exact consumption order
                for c in range(8):
                    r = slice(128 * c, 128 * c + 128)
                    if c == 0:
                        nc.sync.dma_start(out=hwl[c][:, 0:1280],
                                          in_=ht_d[r, 0:1280])
                        nc.sync.dma_start(out=hwl[c][:, 1280:2304],
                                          in_=ht_d[r, 1280:2304])
                    else:
                        nc.sync.dma_start(out=hwl[c], in_=ht_d[r, :])
                nc.sync.dma_start(
                    out=wq, in_=wq_d.rearrange("(c p) n -> p c n", p=128)
                )
                nc.sync.dma_start(
                    out=wv, in_=wv_d.rearrange("(c p) n -> p c n", p=128)
                )
                q2.dma_start(out=smalls, in_=sm_d[:, :])

                # compact copies of ht's global summary columns (matmul
                # operands must have a single free dim, so gather via DVE).
                # ghtB is zero-padded 15 -> 32 so B-score matmuls write full
                # 32-partition strips (pad rows never read downstream).
                ghtA_sb = consts.tile([128, 8, 128], BF16)
                ghtB_sb = consts.tile([128, 8, 32], BF16)
                nc.vector.memset(ghtB_sb, 0.0)
                for c in range(8):
                    src = htl[c].rearrange("p (a b) -> p a b", b=128)
                    nc.vector.tensor_copy(
                        ghtA_sb[:, c, :].rearrange("p (a b) -> p a b", b=8),
                        src[:, :, 120:128],
                    )
                    nc.vector.tensor_copy(ghtB_sb[:, c, 0:15], src[:, 1:16, 0])

                def P(i):
                    # 8 statically-tagged PSUM bank slots, reused across
                    # phases (per-tag WAR deps give fine-grained overlap,
                    # no pool-boundary barrier).
                    return psum.tile([128, 512], F32, tag=f"P{i % 8}",
                                     name=f"P{i % 8}")

                # PE warmup: the tensor engine ramps to full clock only
                # after ~3us of activity. Chew through throwaway matmuls
                # while the first input chunks are still in flight so all
                # real matmuls run at full speed.
                dummy = consts.tile([128, 512], BF16)
                nc.vector.memset(dummy, 0.0)
                warm = P(7)
                for _ in range(5):
                    nc.tensor.matmul(warm, lhsT=dummy[:, 0:128],
                                     rhs=dummy, start=True, stop=True)

                # ---- K then Q projections, chunk-outer over ht ----
                qtl = [consts.tile([128, L], BF16, tag=f"qt{t}", name=f"qt{t}")
                       for t in range(2)]
                ktl = [consts.tile([128, L], BF16, tag=f"kt{t}", name=f"kt{t}")
                       for t in range(2)]
                for dstl, wsrc, bcol in ((ktl, "hw", 2), (qtl, "wq", 0)):
                    if c_outer:
                        tiles = {}
                        for c in range(8):
                            for t in range(2):
                                for n in range(4):
                                    if c == 0:
                                        tiles[t, n] = P(4 * t + n)
                                    lhsT = (
                                        hwl[c][:, 128 * t : 128 * t + 128]
                                        if wsrc == "hw"
                                        else wq[:, c, 128 * t : 128 * t + 128]
                                    )
                                    nc.tensor.matmul(
                                        tiles[t, n],
                                        lhsT=lhsT,
                                        rhs=htl[c][:, 512 * n : 512 * n + 512],
                                        start=(c == 0),
                                        stop=(c == 7),
                                    )
                        for t in range(2):
                            for n in range(4):
                                nc.scalar.activation(
                                    dstl[t][:, 512 * n : 512 * n + 512],
                                    tiles[t, n],
                                    AF.Identity,
                                    bias=bqk[:, bcol + t : bcol + t + 1],
                                )
                    else:
                        for t in range(2):
                            for n in range(4):
                                ps = P(4 * t + n)
                                for c in range(8):
                                    lhsT = (
                                        hwl[c][:, 128 * t : 128 * t + 128]
                                        if wsrc == "hw"
                                        else wq[:, c, 128 * t : 128 * t + 128]
                                    )
                                    nc.tensor.matmul(
                                        ps,
                                        lhsT=lhsT,
                                        rhs=htl[c][:, 512 * n : 512 * n + 512],
                                        start=(c == 0),
                                        stop=(c == 7),
                                    )
                                nc.scalar.activation(
                                    dstl[t][:, 512 * n : 512 * n + 512],
                                    ps,
                                    AF.Identity,
                                    bias=bqk[:, bcol + t : bcol + t + 1],
                                )

                # ---- global gathered K^T and V ----
                ktgA = consts.tile([128, 2, 128], BF16)
                ktgB = consts.tile([128, 2, 128], BF16)
                nc.vector.memset(ktgB.rearrange("p a b -> p (a b)"), 0.0)
                for t in range(2):
                    ps = P(0 + t)
                    for c in range(8):
                        nc.tensor.matmul(
                            ps[:, 0:128],
                            lhsT=hwl[c][:, 128 * t : 128 * t + 128],
                            rhs=ghtA_sb[:, c, :],
                            start=(c == 0), stop=(c == 7),
                        )
                    nc.scalar.activation(
                        ktgA[:, t, :], ps[:, 0:128], AF.Identity,
                        bias=bqk[:, 2 + t : 3 + t],
                    )
                    ps = P(2 + t)
                    for c in range(8):
                        nc.tensor.matmul(
                            ps[:, 0:32],
                            lhsT=hwl[c][:, 128 * t : 128 * t + 128],
                            rhs=ghtB_sb[:, c, :],
                            start=(c == 0), stop=(c == 7),
                        )
                    nc.scalar.activation(
                        ktgB[:, t, 0:32], ps[:, 0:32], AF.Identity,
                        bias=bqk[:, 2 + t : 3 + t],
                    )

                vgA = consts.tile([128, 260], BF16)
                ps = P(4)
                for c in range(8):
                    nc.tensor.matmul(
                        ps[:, 0:260], lhsT=ghtA_sb[:, c, :], rhs=wv[:, c, :],
                        start=(c == 0), stop=(c == 7),
                    )
                nc.vector.tensor_copy(vgA, ps[:, 0:260])
                nc.vector.memset(
                    vgA.rearrange("p (h d) -> p h d", d=65)[:, :, 64:65], 1.0
                )

                vgB = consts.tile([128, 260], BF16)
                ps = P(5)
                for c in range(8):
                    nc.tensor.matmul(
                        ps[0:32, 0:260], lhsT=ghtB_sb[:, c, :], rhs=wv[:, c, :],
                        start=(c == 0), stop=(c == 7),
                    )
                nc.vector.tensor_copy(vgB[0:15, :], ps[0:15, 0:260])
                nc.vector.memset(
                    vgB[0:15, :].rearrange("p (h d) -> p h d", d=65)[:, :, 64:65],
                    1.0,
                )
                for h_ in range(1, 4):
                    nc.sync.dma_start(
                        out=vgB[32 * h_ : 32 * h_ + 15, :], in_=vgB[0:15, :]
                    )

                # ---- attention: per qc, V blocks then scores/exp/PV ----
                vl = [consts.tile([128, 260], BF16, tag=f"v{blk}", name=f"v{blk}")
                      for blk in range(NB)]
                nv = [0]  # rotation counters into the 8 PSUM slots
                ns = [0]
                ncx = [0]
                scores_of = {}

                def emit_pv(qc):
                    # PV + normalize + store, deferred one qc behind scores
                    # so the next qc's exps overlap this PV on the Act engine
                    pB, pAs, pSs = scores_of.pop(qc)
                    outs = [
                        po.tile([128, 256], F32, tag=f"o{j + 1}",
                                name=f"o{j + 1}")
                        for j in range(4)
                    ]
                    cslots = (6, 7) if qc < 2 else (6, 7, 2)
                    for j in range(4):
                        w_ = 4 * qc + j
                        wW = 128 if w_ < 15 else 127
                        cxt = P(cslots[ncx[0] % len(cslots)])
                        ncx[0] += 1
                        for h in range(4):
                            cx = cxt[0:wW, 65 * h : 65 * h + 65]
                            nc.tensor.matmul(
                                cx,
                                lhsT=pAs[h][:, 128 * j : 128 * j + wW],
                                rhs=vgA[:, 65 * h : 65 * h + 65],
                                start=True, stop=False,
                            )
                            nc.tensor.matmul(
                                cx,
                                lhsT=pB[32 * h : 32 * h + 15,
                                        128 * j : 128 * j + wW],
                                rhs=vgB[32 * h : 32 * h + 15,
                                        65 * h : 65 * h + 65],
                                start=False, stop=False,
                                tile_position=(32 * h, 0),
                            )
                            nc.tensor.matmul(
                                cx,
                                lhsT=pSs[h][:, 128 * j : 128 * j + wW],
                                rhs=vl[w_][:, 65 * h : 65 * h + 65],
                                start=False, stop=True,
                            )
                        cxv = cxt[:, 0:260].rearrange("p (h d) -> p h d", d=65)
                        rcp = psmall.tile([128, 4], F32, tag="rcp")
                        nc.vector.reciprocal(rcp[0:wW, :], cxv[0:wW, :, 64])
                        for h in range(4):
                            nc.vector.tensor_scalar_mul(
                                outs[j][0:wW, 64 * h : 64 * h + 64],
                                cxv[0:wW, h, 0:64],
                                rcp[0:wW, h : h + 1],
                            )
                        nc.sync.dma_start(
                            out=out_d[128 * w_ + 1 : 128 * w_ + 1 + wW, :],
                            in_=outs[j][0:wW, :],
                        )

                def emit_V(b0, b1):
                    # V projections (ht is resident; block-inner accumulation)
                    for blk in range(b0, b1):
                        ps = P(nv[0] % 2)
                        nv[0] += 1
                        for c in range(8):
                            nc.tensor.matmul(
                                ps[:, 0:260],
                                lhsT=htl[c][:, 128 * blk : 128 * blk + 128],
                                rhs=wv[:, c, :],
                                start=(c == 0), stop=(c == 7),
                            )
                        nc.vector.tensor_copy(vl[blk], ps[:, 0:260])
                        nc.vector.memset(
                            vl[blk].rearrange("p (h d) -> p h d", d=65)[:, :, 64:65],
                            1.0,
                        )
                        # B-dup exclusion (key col 128*blk, windows >= 1)
                        # by zeroing V row 0: it then adds 0 to numerator AND
                        # denominator, so the S-exp bias is window-invariant
                        # (biasS col 0: just the A-dup rows + attention mask)
                        if blk >= 1:
                            nc.gpsimd.memset(vl[blk][0:1, :], 0.0)

                def emit_scores(qc):
                    lo = 512 * qc + 1
                    W = 512 if qc < 3 else 511
                    slots = (3, 4, 5) if qc < 3 else (0, 1, 3, 4, 5)
                    # B-scores for all 4 heads (packed on partition strips;
                    # unwritten partition strips are never read downstream)
                    pgB = P(2)
                    for h in range(4):
                        t, hh = h // 2, h % 2
                        p0 = 64 * hh
                        nc.tensor.matmul(
                            pgB[32 * h : 32 * h + 32, 0:W],
                            lhsT=ktgB[p0 : p0 + 64, t, 0:32],
                            rhs=qtl[t][p0 : p0 + 64, lo : lo + W],
                            start=True, stop=True,
                            tile_position=(p0, 32 * h),
                        )
                    pB = pp.tile([128, 512], BF16, tag="pB")
                    nc.scalar.activation(pB[:, 0:W], pgB[:, 0:W], AF.Exp, bias=aB)

                    pAs, pSs = [], []
                    for h in range(4):
                        t, hh = h // 2, h % 2
                        p0 = 64 * hh

                        # global-A scores + exp
                        pgA = P(slots[ns[0] % len(slots)])
                        ns[0] += 1
                        nc.tensor.matmul(
                            pgA[:, 0:W],
                            lhsT=ktgA[p0 : p0 + 64, t, :],
                            rhs=qtl[t][p0 : p0 + 64, lo : lo + W],
                            start=True, stop=True,
                        )
                        pA = pp.tile([128, 512], BF16, tag="pA")
                        nc.scalar.activation(pA[:, 0:W], pgA[:, 0:W], AF.Exp,
                                             bias=aA)

                        # local window scores: window w keys = block w
                        pss = P(slots[ns[0] % len(slots)])
                        ns[0] += 1
                        for j in range(4):
                            w_ = 4 * qc + j
                            wW = 128 if w_ < 15 else 127
                            nc.tensor.matmul(
                                pss[:, 128 * j : 128 * j + wW],
                                lhsT=ktl[t][p0 : p0 + 64,
                                            128 * w_ : 128 * w_ + 128],
                                rhs=qtl[t][p0 : p0 + 64,
                                           128 * w_ + 1 : 128 * w_ + 1 + wW],
                                start=True, stop=True,
                            )
                        pS = pp.tile([128, 512], BF16, tag="pS")
                        if am_zero:
                            # row-0 exclusion lives in the zeroed V rows, so
                            # one bias column serves every window
                            nc.scalar.activation(pS[:, 0:W], pss[:, 0:W],
                                                 AF.Exp, bias=bS[:, 0:1])
                        else:
                            for j in range(4):
                                w_ = 4 * qc + j
                                wW = 128 if w_ < 15 else 127
                                nc.scalar.activation(
                                    pS[:, 128 * j : 128 * j + wW],
                                    pss[:, 128 * j : 128 * j + wW],
                                    AF.Exp, bias=bS[:, w_ : w_ + 1],
                                )
                        pAs.append(pA)
                        pSs.append(pS)

                    # dedicated path for query row 0 (keys: block 0 +
                    # globals). Computed 8 queries wide (q=0..7) to avoid
                    # free-size-1 matmuls; only row 0 is stored (rows 1..7
                    # are recomputed by the main window-0 path).
                    if qc == 0 and not skip_q0 and q0_level >= 1:
                        # scores for q=0..7 (only q=0 consumed): all matmuls
                        # use full-128-column weights -- column-tiled PE modes
                        # with narrow rhs hang the hardware.
                        s0 = P(6 + ncx[0] % 2)
                        ncx[0] += 1
                        for h in range(4):
                            t, hh = h // 2, h % 2
                            p0 = 64 * hh
                            q0 = qtl[t][p0 : p0 + 64, 0:8]
                            nc.tensor.matmul(
                                s0[:, 8 * h : 8 * h + 8],
                                lhsT=ktgA[p0 : p0 + 64, t, :], rhs=q0,
                                start=True, stop=True,
                            )
                            if q0_level >= 2:
                                nc.tensor.matmul(
                                    s0[:, 32 + 8 * h : 40 + 8 * h],
                                    lhsT=ktgB[p0 : p0 + 64, t, :], rhs=q0,
                                    start=True, stop=True,
                                )
                            nc.tensor.matmul(
                                s0[:, 64 + 8 * h : 72 + 8 * h],
                                lhsT=ktl[t][p0 : p0 + 64, 0:128], rhs=q0,
                                start=True, stop=True,
                            )
                        # probs, zero-padded to 128 q-columns per head so the
                        # PV matmuls also avoid column tiling
                        pA0 = consts.tile([128, 4, 128], BF16)
                        pB0 = consts.tile([128, 4, 128], BF16)
                        pS0 = consts.tile([128, 4, 128], BF16)
                        if q0_level >= 3:
                            for p_ in (pA0, pB0, pS0):
                                nc.vector.memset(
                                    p_.rearrange("p a b -> p (a b)"), 0.0
                                )
                            for h in range(4):
                                nc.scalar.activation(
                                    pA0[:, h, 0:8], s0[:, 8 * h : 8 * h + 8],
                                    AF.Exp, bias=aA,
                                )
                                if q0_level >= 2:
                                    nc.scalar.activation(
                                        pB0[0:15, h, 0:8],
                                        s0[0:15, 32 + 8 * h : 40 + 8 * h],
                                        AF.Exp, bias=aB[0:15, :],
                                    )
                                nc.scalar.activation(
                                    pS0[:, h, 0:8], s0[:, 64 + 8 * h : 72 + 8 * h],
                                    AF.Exp, bias=bS[:, 0:1],
                                )
                    if qc == 0 and not skip_q0 and q0_level >= 4:
                        ctx0 = P(6 + ncx[0] % 2)
                        ncx[0] += 1
                        for h in range(4):
                            cx = ctx0[:, 65 * h : 65 * h + 65]
                            nc.tensor.matmul(
                                cx, lhsT=pA0[:, h, :],
                                rhs=vgA[:, 65 * h : 65 * h + 65],
                                start=True, stop=False,
                            )
                            nc.tensor.matmul(
                                cx, lhsT=pB0[0:15, h, :],
                                rhs=vgB[0:15, 65 * h : 65 * h + 65],
                                start=False, stop=False,
                            )
                            nc.tensor.matmul(
                                cx, lhsT=pS0[:, h, :],
                                rhs=vl[0][:, 65 * h : 65 * h + 65],
                                start=False, stop=True,
                            )
                    if qc == 0 and not skip_q0 and q0_level >= 5:
                        cxv0 = ctx0[:, 0:260].rearrange("p (h d) -> p h d", d=65)
                        rcp0 = psmall.tile([128, 4], F32, tag="rcp")
                        nc.vector.reciprocal(rcp0[0:8, :], cxv0[0:8, :, 64])
                        out0 = po.tile([128, 256], F32, tag="o0", name="o0")
                        for h in range(4):
                            nc.vector.tensor_scalar_mul(
                                out0[0:8, 64 * h : 64 * h + 64],
                                cxv0[0:8, h, 0:64],
                                rcp0[0:8, h : h + 1],
                            )
                    if qc == 0 and not skip_q0 and q0_level >= 6:
                        nc.sync.dma_start(out=out_d[0:1, :], in_=out0[0:1, :])

                    scores_of[qc] = (pB, pAs, pSs)

                emit_V(0, 5)
                emit_scores(0)
                emit_V(5, 10)
                emit_scores(1)
                emit_pv(0)
                emit_scores(2)
                emit_V(10, 16)
                emit_pv(1)
                emit_scores(3)
                emit_pv(2)
                emit_pv(3)
    nc.finalize()
    return nc


def _prepare_inputs(hidden_states, attention_mask, Wq, bq, Wk, bk, Wv, bv,
                    sparse_mask):
    bf = ml_dtypes.bfloat16
    hs = np.asarray(hidden_states, np.float32)
    am = np.asarray(attention_mask, np.float32).reshape(2, L)
    Wq = np.asarray(Wq, np.float32)
    Wk = np.asarray(Wk, np.float32)
    Wv = np.asarray(Wv, np.float32)
    bq = np.asarray(bq, np.float32)
    bk = np.asarray(bk, np.float32)
    gA, gB = _glob_cols()

    in_maps = []
    per_batch = {}
    for b in range(2):
        ht = hs[b].T.astype(np.float32)  # [1024, 2048]
        # per-window local bias: am over the window's key block, globals
        # excluded (A rows always; k=0 is in B for windows >= 1)
        bS = np.empty((128, NB), np.float32)
        for w in range(NB):
            col = am[b][128 * w : 128 * w + 128].copy()
            col[120:128] = NEG
            if w >= 1:
                col[0] = NEG
            bS[:, w] = col
        per_batch[b] = (
            ht,
            bS,
            am[b][gA].reshape(128, 1).copy(),
            _rep_attnB(am[b][gB]),
        )

    for core in range(8):
        b, g = core // 4, core % 4
        ht, bS, aAv, aBv = per_batch[b]
        cols = slice(256 * g, 256 * g + 256)
        wq = (Wq[:, cols] * 0.125).astype(bf)
        htwk = np.empty((HID, 2304), np.float32)
        htwk[:, 0:256] = Wk[:, cols]
        htwk[:, 256:2304] = ht
        wv_ = np.zeros((HID, 260), np.float32)
        for j in range(4):
            wv_[:, 65 * j : 65 * j + 64] = (
                Wv[:, cols.start + 64 * j : cols.start + 64 * j + 64]
            )
        bqk_ = np.stack(
            [
                bq[cols][:128] * 0.125,
                bq[cols][128:] * 0.125,
                bk[cols][:128],
                bk[cols][128:],
            ],
            axis=1,
        ).astype(np.float32)
        sm = np.empty((128, 6 + NB), np.float32)
        sm[:, 0:4] = bqk_
        sm[:, 4:5] = aAv
        sm[:, 5:6] = aBv
        sm[:, 6 : 6 + NB] = bS
        in_maps.append(
            dict(
                htwk=htwk.astype(bf),
                wq=wq,
                wv=wv_.astype(bf),
                smalls=np.ascontiguousarray(sm),
            )
        )
    # NOTE: bv is folded nowhere: it is zeros by construction in this problem.
    # (If nonzero it would need an input-augmentation row; asserted cheaply.)
    assert np.all(np.asarray(bv) == 0.0), "kernel assumes zero V bias"
    return in_maps


def kernel(hidden_states, attention_mask, Wq, bq, Wk, bk, Wv, bv, sparse_mask,
           trace=False):
    am_zero = bool(np.all(np.asarray(attention_mask) == 0.0))
    key = ("nc", am_zero)
    if key not in _prog_cache:
        _prog_cache[key] = build_program(am_zero=am_zero)
    nc = _prog_cache[key]
    in_maps = _prepare_inputs(
        hidden_states, attention_mask, Wq, bq, Wk, bk, Wv, bv, sparse_mask
    )
    res = run_bass_kernel_spmd(nc, in_maps, list(range(8)), trace=trace)
    out = np.empty((2, L, HID), np.float32)
    for core in range(8):
        b, g = core // 4, core % 4
        out[b][:, 256 * g : 256 * g + 256] = res.results[core]["out"]
    if trace:
        _prog_cache["last_results"] = res
    return out
